# revision 24
# baseline (speedup 1.0000x reference)
"""Multi-head attention (B=8, N=1024, C=1024, H=16) on 8 Trainium2 NeuronCores.

Sharding: pure data-parallel — one batch element per core, weights replicated,
no collectives.

v2 design (vs baseline): bf16 matmul operands everywhere (PSUM accumulation
stays fp32), no DRAM bounce for qk (everything lives in SBUF), per-head-pair
software pipeline so the PE and ACT engines run concurrently, 1024-wide exp
activations (psum tiles spanning 2 banks) to amortize ACT fixed overhead, and
row-packed S matmuls (K=64 head A on array rows 0-63, head B on rows 64-127,
issued back-to-back so they execute concurrently).

Per-core algorithm:
  v-proj:    v[m, dv] natural layout, interleaved [m, 16*(64+1)] with a ones
             column per head (PV then emits softmax denominators for free).
  qk-proj:   per pair p: qp[c(2 heads), n], kp[c, m] bf16 tiles in SBUF.
  attention: per (pair, nt half):
               4 groups: S^T chunks for heads A,B into [128,1024] psum pairs,
               exp (ACT, 1024-wide, scale=1/8) -> eA/eB bf16 [128, 4096]
               PV: U_aug[65, nt] = v_aug.T @ expS accumulated over 8 m-chunks
             U -> SBUF (fp32, frees psum), denominators row 64 -> reciprocal
             -> DRAM bounce -> partition-broadcast -> normalize on GpSimd into
             attn_outT[c, n] bf16.
  out-proj:  out[n, d] = attn_outT.T @ wpT + bias, fp32 out.
"""

import sys

if "/opt/trn_rl_repo" not in sys.path:
    sys.path.insert(0, "/opt/trn_rl_repo")

from contextlib import ExitStack

import numpy as np

import concourse.bass as bass
import concourse.mybir as mybir
from concourse import bacc
import concourse.tile as tile
from concourse import bass_utils

B, N, C, H = 8, 1024, 1024, 16
HD = C // H          # 64
SCALE = HD ** -0.5   # 0.125
P = 128              # SBUF partitions
NT = 512             # moving-dim tile (fp32 PSUM bank limit)
NCH = C // P         # 8 contraction chunks over channels
NMT = N // P         # 8 token tiles of 128
NNT = N // NT        # 2 token tiles of 512
NPAIR = H // 2       # 8 head pairs
F32 = mybir.dt.float32
BF16 = mybir.dt.bfloat16
EXP = mybir.ActivationFunctionType.Exp


def _wait_key(w):
    return (w.sync_type, w.id, w.wait_mode, w.wait_value)


def _weights_sig(ldw):
    a = ldw.ins[0]
    return (a.memref, a.offset, tuple(tuple(x) for x in a.ap), str(a.dtype))


def _optimize_pe_stream(nc):
    """Post-compile peephole pass over the PE instruction stream.

    The compiled stream is LDW,MM,LDW,MM,...; an LDWEIGHTS *between* two
    matmuls serializes their retirement (~46ns extra per MM, and it blocks
    the concurrent execution of row-disjoint K=64 matmul pairs).

    Rule 1 (dedupe): [LDW1, MM1, LDW2, MM2] with identical weight APs ->
      [LDW1, MM1, MM2], folding LDW2's waits into MM2.
    Rule 2 (hoist): [LDW1, MM1, LDW2, MM2] where MM1/MM2 are K=64 row-tiles
      at array rows 0/64 -> [LDW1, LDW2, MM1, MM2] so the two matmuls execute
      concurrently on disjoint row groups. Only done when LDW2's waits are
      implied by LDW1's (same semaphore, same-or-lower threshold), so the
      earlier wait position cannot deadlock.
    """
    import concourse.mybir as mybir

    n_dedupe = n_hoist = 0
    for f in nc.m.functions:
        for blk in f.blocks:
            insts = blk.instructions
            out = []
            i = 0
            n = len(insts)
            cur_sig = None  # weights currently loaded in the PE array
            while i < n:
                a = insts[i]
                if isinstance(a, mybir.InstLdweights):
                    sig = _weights_sig(a)
                    # Rule 1 (running dedupe): the array already holds these
                    # weights and the next instruction is the matmul using
                    # them -> drop the reload, fold its waits into the matmul.
                    if (
                        sig == cur_sig
                        and i + 1 < n
                        and isinstance(insts[i + 1], mybir.InstMatmult)
                    ):
                        mm2 = insts[i + 1]
                        si2 = a.sync_info
                        if si2 is not None and (si2.on_wait or si2.on_update):
                            msi = mm2.sync_info
                            mm2.sync_info = mybir.SyncInfo(
                                on_wait=list(si2.on_wait)
                                + (list(msi.on_wait) if msi else []),
                                on_update=(list(msi.on_update) if msi else [])
                                + list(si2.on_update),
                            )
                        out.append(mm2)
                        i += 2
                        n_dedupe += 1
                        continue
                    # Rule 2 (hoist): [LDW1, MM1@(0,0) K=64, LDW2, MM2@(64,0)
                    # K=64] -> [LDW1, LDW2, MM1, MM2] for concurrent row-tiles.
                    if (
                        i + 3 < n
                        and isinstance(insts[i + 1], mybir.InstMatmult)
                        and isinstance(insts[i + 2], mybir.InstLdweights)
                        and isinstance(insts[i + 3], mybir.InstMatmult)
                    ):
                        ldw1, mm1, ldw2, mm2 = insts[i : i + 4]
                        tp1 = mm1.tile_position
                        tp2 = mm2.tile_position
                        if (
                            tp1 is not None
                            and tp2 is not None
                            and tuple(tp1) == (0, 0)
                            and tuple(tp2) == (64, 0)
                            and ldw1.ins[0].ap[0][1] == 64
                            and ldw2.ins[0].ap[0][1] == 64
                        ):
                            w1 = [
                                _wait_key(w)
                                for w in (
                                    ldw1.sync_info.on_wait if ldw1.sync_info else []
                                )
                            ]
                            w2 = [
                                _wait_key(w)
                                for w in (
                                    ldw2.sync_info.on_wait if ldw2.sync_info else []
                                )
                            ]
                            implied = all(
                                any(
                                    k[0] == kk[0]
                                    and k[1] == kk[1]
                                    and k[2] == kk[2]
                                    and k[3] <= kk[3]
                                    for kk in w1
                                )
                                for k in w2
                            )
                            if implied and not (
                                ldw2.sync_info and ldw2.sync_info.on_update
                            ):
                                out.extend([ldw1, ldw2, mm1, mm2])
                                cur_sig = _weights_sig(ldw2)
                                i += 4
                                n_hoist += 1
                                continue
                    cur_sig = sig
                elif isinstance(a, mybir.InstMatmult):
                    pass  # matmuls don't change the loaded weights
                elif getattr(a, "engine", None) == mybir.EngineType.PE:
                    cur_sig = None  # unknown PE instruction: be conservative
                out.append(a)
                i += 1
            if len(out) != len(insts) or n_dedupe or n_hoist:
                blk.instructions = out
    print(f"_optimize_pe_stream: {n_dedupe} LDW deduped, {n_hoist} LDW hoisted")


def build_module():
    nc = bacc.Bacc("TRN2", target_bir_lowering=False, debug=False, num_devices=B)

    xT = nc.dram_tensor("xT", [C, N], BF16, kind="ExternalInput").ap()
    wqkT = nc.dram_tensor("wqkT", [C, 2 * C], BF16, kind="ExternalInput").ap()
    wvT = nc.dram_tensor("wvT", [C, C], BF16, kind="ExternalInput").ap()
    wpT = nc.dram_tensor("wpT", [C, C], BF16, kind="ExternalInput").ap()
    bias = nc.dram_tensor("bias_bc", [P, C], F32, kind="ExternalInput").ap()
    ones_col = nc.dram_tensor("ones_col", [P, H], BF16, kind="ExternalInput").ap()
    out = nc.dram_tensor("out", [N, C], F32, kind="ExternalOutput").ap()

    with tile.TileContext(nc) as tc, ExitStack() as ctx:
        dram = ctx.enter_context(tc.tile_pool(name="dram", bufs=1, space="DRAM"))
        rden_d = dram.tile([H * NNT, NT], F32, tag="rden_d", name="rden_d")

        xt_pool = ctx.enter_context(tc.tile_pool(name="xt", bufs=8))
        qk_pool = ctx.enter_context(tc.tile_pool(name="qk", bufs=6))
        v_pool = ctx.enter_context(tc.tile_pool(name="v", bufs=8))
        e_pool = ctx.enter_context(tc.tile_pool(name="e", bufs=4))
        u_pool = ctx.enter_context(tc.tile_pool(name="u", bufs=6))
        aot_pool = ctx.enter_context(tc.tile_pool(name="aot", bufs=1))
        w_pool = ctx.enter_context(tc.tile_pool(name="wst", bufs=9))
        wqk_pool = ctx.enter_context(tc.tile_pool(name="wqk", bufs=24))
        den_pool = ctx.enter_context(tc.tile_pool(name="den", bufs=4))
        rbc_pool = ctx.enter_context(tc.tile_pool(name="rbc", bufs=6))
        one_pool = ctx.enter_context(tc.tile_pool(name="one", bufs=1))
        stage_pool = ctx.enter_context(tc.tile_pool(name="stage", bufs=3))
        # One [128,1024] (2-bank) psum tag shared by S, v-proj, qk-proj and
        # out-proj: slot grants follow emission order, so the interleave of
        # projection blocks with the S ping-pong is under our control.
        # 3 slots = 6 banks; + 2 banks for PV below = all 8.
        big_psum = ctx.enter_context(tc.tile_pool(name="big_ps", bufs=3, space="PSUM"))
        pv_psum = ctx.enter_context(tc.tile_pool(name="pv_ps", bufs=2, space="PSUM"))
        s_psum = big_psum
        pj_psum = big_psum

        # ---------- input loads ----------
        xts = []
        for t in range(NCH):
            xt_t = xt_pool.tile([P, N], BF16, tag="xt", name=f"xt{t}")
            nc.sync.dma_start(xt_t, xT[t * P : (t + 1) * P, :])
            xts.append(xt_t)
        bias_sb = one_pool.tile([P, C], F32, tag="bias", name="bias_sb")
        nc.sync.dma_start(bias_sb, bias)
        vsb = []
        for mt in range(NMT):
            v_t = v_pool.tile([P, H * (HD + 1)], BF16, tag="v", name=f"v{mt}")
            nc.sync.dma_start(
                v_t.rearrange("p (h w) -> p h w", w=HD + 1)[:, :, HD : HD + 1], ones_col
            )
            vsb.append(v_t)
        aot = [
            aot_pool.tile([P, N], BF16, tag=f"aot{t}", name=f"aot{t}")
            for t in range(NCH)
        ]

        # ---------- v projection (natural layout + ones cols) ----------
        # One [128,1024] psum block per m-tile; the two dvt halves pair on the
        # same stationary operand (xts chunk) so both matmuls become ready
        # together and the duplicate LDWEIGHTS can be dropped. Blocks are
        # emitted interleaved into the first pairs' S chains (3rd psum slot).
        wv_tiles = []

        def load_wv():
            for ck in range(NCH):
                wv_t = w_pool.tile([P, N], BF16, tag="wst", name=f"wv{ck}")
                nc.sync.dma_start(wv_t, wvT[ck * P : (ck + 1) * P, :])
                wv_tiles.append(wv_t)

        def emit_vblock(mt):
            ps = pj_psum.tile([P, 2 * NT], F32, tag="big", name=f"psv{mt}")
            for ck in range(NCH):
                for dvt in range(NNT):
                    nc.tensor.matmul(
                        ps[:, dvt * NT : (dvt + 1) * NT],
                        lhsT=xts[ck][:, mt * P : (mt + 1) * P],
                        rhs=wv_tiles[ck][:, dvt * NT : (dvt + 1) * NT],
                        start=(ck == 0),
                        stop=(ck == NCH - 1),
                    )
            nc.vector.tensor_copy(
                vsb[mt].rearrange("p (h w) -> p h w", w=HD + 1)[:, :, 0:HD],
                ps.rearrange("p (h w) -> p h w", w=HD),
            )

        # ---------- qk projection for one head pair, SBUF-resident ----------
        def emit_qkproj(p):
            qp = qk_pool.tile([P, N], BF16, tag="qk", name=f"qp{p}")
            kp = qk_pool.tile([P, N], BF16, tag="qk", name=f"kp{p}")
            for which, col0, dstt in ((0, p * P, qp), (1, C + p * P, kp)):
                wts = []
                for ck in range(NCH):
                    w_t = wqk_pool.tile(
                        [P, P], BF16, tag="wqk", name=f"w{which}_{p}_{ck}"
                    )
                    nc.sync.dma_start(w_t, wqkT[ck * P : (ck + 1) * P, col0 : col0 + P])
                    wts.append(w_t)
                ps = pj_psum.tile([P, 2 * NT], F32, tag="big", name=f"psqk{which}_{p}")
                for ck in range(NCH):
                    for nt_ in range(NNT):
                        nc.tensor.matmul(
                            ps[:, nt_ * NT : (nt_ + 1) * NT],
                            lhsT=wts[ck],
                            rhs=xts[ck][:, nt_ * NT : (nt_ + 1) * NT],
                            start=(ck == 0),
                            stop=(ck == NCH - 1),
                        )
                nc.vector.tensor_copy(dstt, ps)
            return qp, kp

        # ---------- attention ----------
        pair_units = {}

        def emit_denorm(p, punits):
            den_g = den_pool.tile([4, NT], F32, tag="den", name=f"den{p}")
            for i, (h, nt_, u_t) in enumerate(punits):
                nc.sync.dma_start(den_g[i : i + 1, :], u_t[HD : HD + 1, :])
            rden = den_pool.tile([4, NT], F32, tag="rden", name=f"rden{p}")
            nc.vector.reciprocal_approx_fast(out=rden, in_=den_g)
            nc.sync.dma_start(rden_d[p * 4 : p * 4 + 4, :], rden)
            for i, (h, nt_, u_t) in enumerate(punits):
                rbc = rbc_pool.tile([HD, NT], F32, tag="rbc", name=f"rbc{h}_{nt_}")
                src_ = rden_d[p * 4 + i : p * 4 + i + 1, :]
                bsrc = bass.AP(
                    tensor=src_.tensor,
                    offset=src_.offset,
                    ap=[[0, HD], list(src_.ap[-1])],
                )
                nc.sync.dma_start(out=rbc, in_=bsrc)
                ct, prow = h // 2, (h % 2) * HD
                nc.gpsimd.tensor_mul(
                    aot[ct][prow : prow + HD, nt_ * NT : (nt_ + 1) * NT],
                    u_t[0:HD, :],
                    rbc,
                )

        def emit_s_exp_nt(p, nt_, qp, kp, extras=()):
            """S^T + exp for one (pair, n-half). Heads A and B share one
            [128, 1024] psum tile per m-chunk (A in the low bank, B in the
            high bank) so both matmuls become ready together; the post-compile
            pass hoists B's LDWEIGHTS above A's matmul, making the two K=64
            matmuls (array rows 0-63 / 64-127) run concurrently.
            `extras` are projection-block emitters interleaved mid-chain: they
            take the 3rd psum slot while the S chain ping-pongs the other two.
            Returns e tile [128, 8192] laid out as [A_mc0|B_mc0|A_mc1|...]."""
            e_t = e_pool.tile([P, 2 * NMT * NT], BF16, tag="e", name=f"e{p}_{nt_}")
            extras = list(extras)
            for mc in range(NMT):
                if extras and mc in (2, 5):
                    extras.pop(0)()
                s_t = s_psum.tile([P, 2 * NT], F32, tag="big", name=f"s{p}_{nt_}_{mc}")
                # high priority: the S pair feeds ACT (the attention-phase
                # pacer) and must pop back-to-back so the post-compile hoist
                # can make the two K=64 row-tiles run concurrently.
                with tc.high_priority():
                    nc.tensor.matmul(
                        s_t[:, 0:NT],
                        lhsT=kp[0:HD, mc * P : (mc + 1) * P],
                        rhs=qp[0:HD, nt_ * NT : (nt_ + 1) * NT],
                        start=True,
                        stop=True,
                    )
                    nc.tensor.matmul(
                        s_t[:, NT : 2 * NT],
                        lhsT=kp[HD:P, mc * P : (mc + 1) * P],
                        rhs=qp[HD:P, nt_ * NT : (nt_ + 1) * NT],
                        start=True,
                        stop=True,
                    )
                nc.scalar.activation(
                    e_t[:, mc * 2 * NT : (mc + 1) * 2 * NT], s_t, EXP, scale=SCALE
                )
            return e_t

        def emit_pv(p, h, e_nt0, e_nt1):
            """PV for one head over both n-halves, paired on the stationary
            v_aug chunk so the duplicate LDWEIGHTS can be dropped."""
            j = h % 2
            pss = [
                pv_psum.tile([HD + 1, NT], F32, tag="pv", name=f"pu{h}_{nt_}")
                for nt_ in range(NNT)
            ]
            for mc in range(NMT):
                for nt_, e_t in ((0, e_nt0), (1, e_nt1)):
                    nc.tensor.matmul(
                        pss[nt_],
                        lhsT=vsb[mc][:, h * (HD + 1) : (h + 1) * (HD + 1)],
                        rhs=e_t[:, (mc * 2 + j) * NT : (mc * 2 + j + 1) * NT],
                        start=(mc == 0),
                        stop=(mc == NMT - 1),
                    )
            for nt_ in range(NNT):
                u_t = u_pool.tile([HD + 1, NT], F32, tag="u", name=f"u{h}_{nt_}")
                nc.vector.tensor_copy(u_t, pss[nt_])
                pair_units.setdefault(p, []).append((h, nt_, u_t))
                if len(pair_units[p]) == 4:
                    emit_denorm(p, pair_units.pop(p))

        # ---------- output projection + bias ----------
        # dt halves paired on the stationary aot chunk, single [128, 1024]
        # psum per n-tile; alternate between the pj and s psum pools (the s
        # pool is free by the tail) to keep the tail double-buffered.
        def emit_outproj():
            wp_tiles = []
            for ck in range(NCH):
                wp_t = w_pool.tile([P, N], BF16, tag="wst", name=f"wp{ck}")
                nc.sync.dma_start(wp_t, wpT[ck * P : (ck + 1) * P, :])
                wp_tiles.append(wp_t)
            for nt2 in range(NMT):
                ps = pj_psum.tile([P, 2 * NT], F32, tag="big", name=f"pso{nt2}")
                for ck in range(NCH):
                    for dt in range(NNT):
                        nc.tensor.matmul(
                            ps[:, dt * NT : (dt + 1) * NT],
                            lhsT=aot[ck][:, nt2 * P : (nt2 + 1) * P],
                            rhs=wp_tiles[ck][:, dt * NT : (dt + 1) * NT],
                            start=(ck == 0),
                            stop=(ck == NCH - 1),
                        )
                o_sb = stage_pool.tile([P, 2 * NT], F32, tag="stage", name=f"o{nt2}")
                nc.vector.tensor_add(o_sb, ps, bias_sb)
                nc.sync.dma_start(out[nt2 * P : (nt2 + 1) * P, :], o_sb)

        # ---------- pipeline ----------
        load_wv()
        qks = {0: emit_qkproj(0)}
        qks[1] = emit_qkproj(1)
        # v-proj blocks ride the 3rd psum slot inside pairs 0-1's S chains.
        vblocks = [[0, 1], [2, 3], [4, 5], [6, 7]]
        for p in range(NPAIR):
            qp, kp = qks.pop(p)
            ex0 = [lambda mt=mt: emit_vblock(mt) for mt in vblocks[2 * p]] if p < 2 else ()
            ex1 = [lambda mt=mt: emit_vblock(mt) for mt in vblocks[2 * p + 1]] if p < 2 else ()
            e0 = emit_s_exp_nt(p, 0, qp, kp, ex0)
            e1 = emit_s_exp_nt(p, 1, qp, kp, ex1)
            emit_pv(p, 2 * p, e0, e1)
            emit_pv(p, 2 * p + 1, e0, e1)
            if p + 2 < NPAIR:
                qks[p + 2] = emit_qkproj(p + 2)
        emit_outproj()

    nc.compile()
    import os

    if not os.environ.get("K_NOOPT"):
        _optimize_pe_stream(nc)
    return nc


def make_in_maps(x, w_qkv, w_proj, b_proj):
    import ml_dtypes

    bf16 = ml_dtypes.bfloat16
    wqkT = np.ascontiguousarray(w_qkv[: 2 * C].T.astype(bf16))
    wvT = np.ascontiguousarray(w_qkv[2 * C :].T.astype(bf16))
    wpT = np.ascontiguousarray(w_proj.T.astype(bf16))
    bias_bc = np.ascontiguousarray(
        np.broadcast_to(b_proj, (P, C)).astype(np.float32)
    )
    ones = np.ones((P, H), dtype=bf16)
    in_maps = []
    for b in range(B):
        in_maps.append(
            {
                "xT": np.ascontiguousarray(x[b].T.astype(bf16)),
                "wqkT": wqkT,
                "wvT": wvT,
                "wpT": wpT,
                "bias_bc": bias_bc,
                "ones_col": ones,
            }
        )
    return in_maps


_CACHED_NC = None


def kernel(x, w_qkv, w_proj, b_proj):
    global _CACHED_NC
    x = np.asarray(x, dtype=np.float32)
    w_qkv = np.asarray(w_qkv, dtype=np.float32)
    w_proj = np.asarray(w_proj, dtype=np.float32)
    b_proj = np.asarray(b_proj, dtype=np.float32)
    if _CACHED_NC is None:
        _CACHED_NC = build_module()
    nc = _CACHED_NC
    in_maps = make_in_maps(x, w_qkv, w_proj, b_proj)
    res = bass_utils.run_bass_kernel_spmd(nc, in_maps, core_ids=list(range(B)))
    return np.stack([res.results[b]["out"] for b in range(B)], axis=0)


if __name__ == "__main__":
    nc = build_module()
    ninst = sum(len(b.instructions) for b in nc.m.functions[0].blocks)
    print("module built ok;", ninst, "instructions")


# revision 35
# speedup vs baseline: 9824.0032x; 9824.0032x over previous
"""Multi-head attention (B=8, N=1024, C=1024, H=16) on 8 Trainium2 NeuronCores.

Sharding: pure data-parallel — one batch element per core, weights replicated,
no collectives.

v2 design (vs baseline): bf16 matmul operands everywhere (PSUM accumulation
stays fp32), no DRAM bounce for qk (everything lives in SBUF), per-head-pair
software pipeline so the PE and ACT engines run concurrently, 1024-wide exp
activations (psum tiles spanning 2 banks) to amortize ACT fixed overhead, and
row-packed S matmuls (K=64 head A on array rows 0-63, head B on rows 64-127,
issued back-to-back so they execute concurrently).

Per-core algorithm:
  v-proj:    v[m, dv] natural layout, interleaved [m, 16*(64+1)] with a ones
             column per head (PV then emits softmax denominators for free).
  qk-proj:   per pair p: qp[c(2 heads), n], kp[c, m] bf16 tiles in SBUF.
  attention: per (pair, nt half):
               4 groups: S^T chunks for heads A,B into [128,1024] psum pairs,
               exp (ACT, 1024-wide, scale=1/8) -> eA/eB bf16 [128, 4096]
               PV: U_aug[65, nt] = v_aug.T @ expS accumulated over 8 m-chunks
             U -> SBUF (fp32, frees psum), denominators row 64 -> reciprocal
             -> DRAM bounce -> partition-broadcast -> normalize on GpSimd into
             attn_outT[c, n] bf16.
  out-proj:  out[n, d] = attn_outT.T @ wpT + bias, fp32 out.
"""

import sys

if "/opt/trn_rl_repo" not in sys.path:
    sys.path.insert(0, "/opt/trn_rl_repo")

from contextlib import ExitStack

import numpy as np

import concourse.bass as bass
import concourse.mybir as mybir
from concourse import bacc
import concourse.tile as tile
from concourse import bass_utils

B, N, C, H = 8, 1024, 1024, 16
HD = C // H          # 64
SCALE = HD ** -0.5   # 0.125
P = 128              # SBUF partitions
NT = 512             # moving-dim tile (fp32 PSUM bank limit)
NCH = C // P         # 8 contraction chunks over channels
NMT = N // P         # 8 token tiles of 128
NNT = N // NT        # 2 token tiles of 512
NPAIR = H // 2       # 8 head pairs
F32 = mybir.dt.float32
BF16 = mybir.dt.bfloat16
EXP = mybir.ActivationFunctionType.Exp


def _wait_key(w):
    return (w.sync_type, w.id, w.wait_mode, w.wait_value)


def _weights_sig(ldw):
    a = ldw.ins[0]
    return (a.memref, a.offset, tuple(tuple(x) for x in a.ap), str(a.dtype))


def _optimize_pe_stream(nc):
    """Post-compile peephole pass over the PE instruction stream.

    The compiled stream is LDW,MM,LDW,MM,...; an LDWEIGHTS *between* two
    matmuls serializes their retirement (~46ns extra per MM, and it blocks
    the concurrent execution of row-disjoint K=64 matmul pairs).

    Rule 1 (dedupe): [LDW1, MM1, LDW2, MM2] with identical weight APs ->
      [LDW1, MM1, MM2], folding LDW2's waits into MM2.
    Rule 2 (hoist): [LDW1, MM1, LDW2, MM2] where MM1/MM2 are K=64 row-tiles
      at array rows 0/64 -> [LDW1, LDW2, MM1, MM2] so the two matmuls execute
      concurrently on disjoint row groups. Only done when LDW2's waits are
      implied by LDW1's (same semaphore, same-or-lower threshold), so the
      earlier wait position cannot deadlock.
    """
    import concourse.mybir as mybir

    n_dedupe = n_hoist = 0
    for f in nc.m.functions:
        for blk in f.blocks:
            insts = blk.instructions
            out = []
            i = 0
            n = len(insts)
            cur_sig = None  # weights currently loaded in the PE array
            # Guards against deleting the *guarded* second load of a compiler
            # [LDW, LDW, MM, MM] prefetch pair: only dedupe once a matmul has
            # consumed the current load.
            cur_consumed = False
            while i < n:
                a = insts[i]
                if isinstance(a, mybir.InstLdweights):
                    sig = _weights_sig(a)
                    # Rule 1 (running dedupe): the array already holds these
                    # weights and the next instruction is the matmul using
                    # them -> drop the redundant reload. A wait can only ride
                    # on an LDWEIGHTS (the PE hw-decoder ignores waits on
                    # MATMUL), so only wait-free duplicate LDWs are deletable.
                    if (
                        sig == cur_sig
                        and cur_consumed
                        and (
                            a.sync_info is None
                            or not (a.sync_info.on_wait or a.sync_info.on_update)
                        )
                        and i + 1 < n
                        and isinstance(insts[i + 1], mybir.InstMatmult)
                    ):
                        out.append(insts[i + 1])
                        i += 2
                        n_dedupe += 1
                        cur_consumed = True
                        continue
                    # Rule 2 (hoist): [LDW1, MM1@(0,0) K=64, LDW2, MM2@(64,0)
                    # K=64] -> [LDW1, LDW2, MM1, MM2] for concurrent row-tiles.
                    if (
                        i + 3 < n
                        and isinstance(insts[i + 1], mybir.InstMatmult)
                        and isinstance(insts[i + 2], mybir.InstLdweights)
                        and isinstance(insts[i + 3], mybir.InstMatmult)
                    ):
                        ldw1, mm1, ldw2, mm2 = insts[i : i + 4]
                        tp1 = mm1.tile_position
                        tp2 = mm2.tile_position
                        if (
                            tp1 is not None
                            and tp2 is not None
                            and tuple(tp1) == (0, 0)
                            and tuple(tp2) == (64, 0)
                            and ldw1.ins[0].ap[0][1] == 64
                            and ldw2.ins[0].ap[0][1] == 64
                        ):
                            w1 = [
                                _wait_key(w)
                                for w in (
                                    ldw1.sync_info.on_wait if ldw1.sync_info else []
                                )
                            ]
                            w2 = [
                                _wait_key(w)
                                for w in (
                                    ldw2.sync_info.on_wait if ldw2.sync_info else []
                                )
                            ]
                            implied = all(
                                any(
                                    k[0] == kk[0]
                                    and k[1] == kk[1]
                                    and k[2] == kk[2]
                                    and k[3] <= kk[3]
                                    for kk in w1
                                )
                                for k in w2
                            )
                            if implied and not (
                                ldw2.sync_info and ldw2.sync_info.on_update
                            ):
                                out.extend([ldw1, ldw2, mm1, mm2])
                                cur_sig = _weights_sig(ldw2)
                                cur_consumed = True
                                i += 4
                                n_hoist += 1
                                continue
                    cur_sig = sig
                    cur_consumed = False
                elif isinstance(a, mybir.InstMatmult):
                    cur_consumed = True  # matmuls consume the loaded weights
                elif getattr(a, "engine", None) == mybir.EngineType.PE:
                    cur_sig = None  # unknown PE instruction: be conservative
                out.append(a)
                i += 1
            if len(out) != len(insts) or n_dedupe or n_hoist:
                blk.instructions = out
    print(f"_optimize_pe_stream: {n_dedupe} LDW deduped, {n_hoist} LDW hoisted")


def build_module():
    nc = bacc.Bacc("TRN2", target_bir_lowering=False, debug=False, num_devices=B)

    xT = nc.dram_tensor("xT", [C, N], BF16, kind="ExternalInput").ap()
    wqkT = nc.dram_tensor("wqkT", [C, 2 * C], BF16, kind="ExternalInput").ap()
    wvT = nc.dram_tensor("wvT", [C, C], BF16, kind="ExternalInput").ap()
    wpT = nc.dram_tensor("wpT", [C, C], BF16, kind="ExternalInput").ap()
    bias = nc.dram_tensor("bias_bc", [P, C], F32, kind="ExternalInput").ap()
    ones_col = nc.dram_tensor("ones_col", [P, H], BF16, kind="ExternalInput").ap()
    out = nc.dram_tensor("out", [N, C], F32, kind="ExternalOutput").ap()

    with tile.TileContext(nc) as tc, ExitStack() as ctx:
        dram = ctx.enter_context(tc.tile_pool(name="dram", bufs=1, space="DRAM"))
        rden_d = dram.tile([H * NNT, NT], F32, tag="rden_d", name="rden_d")

        xt_pool = ctx.enter_context(tc.tile_pool(name="xt", bufs=8))
        qk_pool = ctx.enter_context(tc.tile_pool(name="qk", bufs=6))
        v_pool = ctx.enter_context(tc.tile_pool(name="v", bufs=8))
        e_pool = ctx.enter_context(tc.tile_pool(name="e", bufs=4))
        u_pool = ctx.enter_context(tc.tile_pool(name="u", bufs=6))
        aot_pool = ctx.enter_context(tc.tile_pool(name="aot", bufs=1))
        w_pool = ctx.enter_context(tc.tile_pool(name="wst", bufs=9))
        wqk_pool = ctx.enter_context(tc.tile_pool(name="wqk", bufs=24))
        den_pool = ctx.enter_context(tc.tile_pool(name="den", bufs=4))
        rbc_pool = ctx.enter_context(tc.tile_pool(name="rbc", bufs=6))
        one_pool = ctx.enter_context(tc.tile_pool(name="one", bufs=1))
        stage_pool = ctx.enter_context(tc.tile_pool(name="stage", bufs=3))
        s_psum = ctx.enter_context(tc.tile_pool(name="s_ps", bufs=2, space="PSUM"))
        pv_psum = ctx.enter_context(tc.tile_pool(name="pv_ps", bufs=2, space="PSUM"))
        pj_psum = ctx.enter_context(tc.tile_pool(name="pj_ps", bufs=1, space="PSUM"))

        # ---------- input loads ----------
        xts = []
        for t in range(NCH):
            xt_t = xt_pool.tile([P, N], BF16, tag="xt", name=f"xt{t}")
            nc.sync.dma_start(xt_t, xT[t * P : (t + 1) * P, :])
            xts.append(xt_t)
        bias_sb = one_pool.tile([P, C], F32, tag="bias", name="bias_sb")
        nc.sync.dma_start(bias_sb, bias)
        vsb = []
        for mt in range(NMT):
            v_t = v_pool.tile([P, H * (HD + 1)], BF16, tag="v", name=f"v{mt}")
            nc.sync.dma_start(
                v_t.rearrange("p (h w) -> p h w", w=HD + 1)[:, :, HD : HD + 1], ones_col
            )
            vsb.append(v_t)
        aot = [
            aot_pool.tile([P, N], BF16, tag=f"aot{t}", name=f"aot{t}")
            for t in range(NCH)
        ]

        # ---------- v projection (natural layout + ones cols) ----------
        # dvt halves write the two banks of one [128, 1024] psum tile and pair
        # on the same stationary operand (xts chunk) so both matmuls become
        # ready together and the duplicate LDWEIGHTS can be dropped.
        def emit_vproj():
            wv_tiles = []
            for ck in range(NCH):
                wv_t = w_pool.tile([P, N], BF16, tag="wst", name=f"wv{ck}")
                nc.sync.dma_start(wv_t, wvT[ck * P : (ck + 1) * P, :])
                wv_tiles.append(wv_t)
            for mt in range(NMT):
                ps = pj_psum.tile([P, 2 * NT], F32, tag="pj", name=f"psv{mt}")
                for ck in range(NCH):
                    for dvt in range(NNT):
                        nc.tensor.matmul(
                            ps[:, dvt * NT : (dvt + 1) * NT],
                            lhsT=xts[ck][:, mt * P : (mt + 1) * P],
                            rhs=wv_tiles[ck][:, dvt * NT : (dvt + 1) * NT],
                            start=(ck == 0),
                            stop=(ck == NCH - 1),
                        )
                nc.vector.tensor_copy(
                    vsb[mt].rearrange("p (h w) -> p h w", w=HD + 1)[:, :, 0:HD],
                    ps.rearrange("p (h w) -> p h w", w=HD),
                )

        # ---------- qk projection for one head pair, SBUF-resident ----------
        def emit_qkproj(p):
            qp = qk_pool.tile([P, N], BF16, tag="qk", name=f"qp{p}")
            kp = qk_pool.tile([P, N], BF16, tag="qk", name=f"kp{p}")
            for which, col0, dstt in ((0, p * P, qp), (1, C + p * P, kp)):
                wts = []
                for ck in range(NCH):
                    w_t = wqk_pool.tile(
                        [P, P], BF16, tag="wqk", name=f"w{which}_{p}_{ck}"
                    )
                    nc.sync.dma_start(w_t, wqkT[ck * P : (ck + 1) * P, col0 : col0 + P])
                    wts.append(w_t)
                ps = pj_psum.tile([P, 2 * NT], F32, tag="pj", name=f"psqk{which}_{p}")
                for ck in range(NCH):
                    for nt_ in range(NNT):
                        nc.tensor.matmul(
                            ps[:, nt_ * NT : (nt_ + 1) * NT],
                            lhsT=wts[ck],
                            rhs=xts[ck][:, nt_ * NT : (nt_ + 1) * NT],
                            start=(ck == 0),
                            stop=(ck == NCH - 1),
                        )
                nc.vector.tensor_copy(dstt, ps)
            return qp, kp

        # ---------- attention ----------
        pair_units = {}

        def emit_denorm(p, punits):
            den_g = den_pool.tile([4, NT], F32, tag="den", name=f"den{p}")
            for i, (h, nt_, u_t) in enumerate(punits):
                nc.sync.dma_start(den_g[i : i + 1, :], u_t[HD : HD + 1, :])
            rden = den_pool.tile([4, NT], F32, tag="rden", name=f"rden{p}")
            nc.vector.reciprocal_approx_fast(out=rden, in_=den_g)
            nc.sync.dma_start(rden_d[p * 4 : p * 4 + 4, :], rden)
            for i, (h, nt_, u_t) in enumerate(punits):
                rbc = rbc_pool.tile([HD, NT], F32, tag="rbc", name=f"rbc{h}_{nt_}")
                src_ = rden_d[p * 4 + i : p * 4 + i + 1, :]
                bsrc = bass.AP(
                    tensor=src_.tensor,
                    offset=src_.offset,
                    ap=[[0, HD], list(src_.ap[-1])],
                )
                nc.sync.dma_start(out=rbc, in_=bsrc)
                ct, prow = h // 2, (h % 2) * HD
                nc.gpsimd.tensor_mul(
                    aot[ct][prow : prow + HD, nt_ * NT : (nt_ + 1) * NT],
                    u_t[0:HD, :],
                    rbc,
                )

        def emit_s_exp_nt(p, nt_, qp, kp):
            """S^T + exp for one (pair, n-half). Heads A and B share one
            [128, 1024] psum tile per m-chunk (A in the low bank, B in the
            high bank) so both matmuls become ready together; the post-compile
            pass hoists B's LDWEIGHTS above A's matmul, making the two K=64
            matmuls (array rows 0-63 / 64-127) run concurrently.
            Returns e tile [128, 8192] laid out as [A_mc0|B_mc0|A_mc1|...]."""
            e_t = e_pool.tile([P, 2 * NMT * NT], BF16, tag="e", name=f"e{p}_{nt_}")
            for mc in range(NMT):
                s_t = s_psum.tile([P, 2 * NT], F32, tag="s", name=f"s{p}_{nt_}_{mc}")
                # high priority: the S pair feeds ACT (the attention-phase
                # pacer) and must pop back-to-back so the post-compile hoist
                # can make the two K=64 row-tiles run concurrently.
                with tc.high_priority():
                    nc.tensor.matmul(
                        s_t[:, 0:NT],
                        lhsT=kp[0:HD, mc * P : (mc + 1) * P],
                        rhs=qp[0:HD, nt_ * NT : (nt_ + 1) * NT],
                        start=True,
                        stop=True,
                    )
                    nc.tensor.matmul(
                        s_t[:, NT : 2 * NT],
                        lhsT=kp[HD:P, mc * P : (mc + 1) * P],
                        rhs=qp[HD:P, nt_ * NT : (nt_ + 1) * NT],
                        start=True,
                        stop=True,
                    )
                nc.scalar.activation(
                    e_t[:, mc * 2 * NT : (mc + 1) * 2 * NT], s_t, EXP, scale=SCALE
                )
            return e_t

        def emit_pv(p, h, e_nt0, e_nt1):
            """PV for one head over both n-halves, paired on the stationary
            v_aug chunk so the duplicate LDWEIGHTS can be dropped."""
            j = h % 2
            pss = [
                pv_psum.tile([HD + 1, NT], F32, tag="pv", name=f"pu{h}_{nt_}")
                for nt_ in range(NNT)
            ]
            for mc in range(NMT):
                for nt_, e_t in ((0, e_nt0), (1, e_nt1)):
                    nc.tensor.matmul(
                        pss[nt_],
                        lhsT=vsb[mc][:, h * (HD + 1) : (h + 1) * (HD + 1)],
                        rhs=e_t[:, (mc * 2 + j) * NT : (mc * 2 + j + 1) * NT],
                        start=(mc == 0),
                        stop=(mc == NMT - 1),
                    )
            for nt_ in range(NNT):
                u_t = u_pool.tile([HD + 1, NT], F32, tag="u", name=f"u{h}_{nt_}")
                nc.vector.tensor_copy(u_t, pss[nt_])
                pair_units.setdefault(p, []).append((h, nt_, u_t))
                if len(pair_units[p]) == 4:
                    emit_denorm(p, pair_units.pop(p))

        # ---------- output projection + bias ----------
        # dt halves paired on the stationary aot chunk, single [128, 1024]
        # psum per n-tile; alternate between the pj and s psum pools (the s
        # pool is free by the tail) to keep the tail double-buffered.
        def emit_outproj():
            wp_tiles = []
            for ck in range(NCH):
                wp_t = w_pool.tile([P, N], BF16, tag="wst", name=f"wp{ck}")
                nc.sync.dma_start(wp_t, wpT[ck * P : (ck + 1) * P, :])
                wp_tiles.append(wp_t)
            for nt2 in range(NMT):
                pool = pj_psum if nt2 % 2 == 0 else s_psum
                ps = pool.tile(
                    [P, 2 * NT], F32, tag="pj" if nt2 % 2 == 0 else "s",
                    name=f"pso{nt2}",
                )
                for ck in range(NCH):
                    for dt in range(NNT):
                        nc.tensor.matmul(
                            ps[:, dt * NT : (dt + 1) * NT],
                            lhsT=aot[ck][:, nt2 * P : (nt2 + 1) * P],
                            rhs=wp_tiles[ck][:, dt * NT : (dt + 1) * NT],
                            start=(ck == 0),
                            stop=(ck == NCH - 1),
                        )
                o_sb = stage_pool.tile([P, 2 * NT], F32, tag="stage", name=f"o{nt2}")
                nc.vector.tensor_add(o_sb, ps, bias_sb)
                nc.sync.dma_start(out[nt2 * P : (nt2 + 1) * P, :], o_sb)

        # ---------- pipeline ----------
        qks = {0: emit_qkproj(0)}
        qks[1] = emit_qkproj(1)
        emit_vproj()
        for p in range(NPAIR):
            qp, kp = qks.pop(p)
            e0 = emit_s_exp_nt(p, 0, qp, kp)
            e1 = emit_s_exp_nt(p, 1, qp, kp)
            emit_pv(p, 2 * p, e0, e1)
            emit_pv(p, 2 * p + 1, e0, e1)
            if p + 2 < NPAIR:
                qks[p + 2] = emit_qkproj(p + 2)
        emit_outproj()

    nc.compile()
    _optimize_pe_stream(nc)
    return nc


def make_in_maps(x, w_qkv, w_proj, b_proj):
    import ml_dtypes

    bf16 = ml_dtypes.bfloat16
    wqkT = np.ascontiguousarray(w_qkv[: 2 * C].T.astype(bf16))
    wvT = np.ascontiguousarray(w_qkv[2 * C :].T.astype(bf16))
    wpT = np.ascontiguousarray(w_proj.T.astype(bf16))
    bias_bc = np.ascontiguousarray(
        np.broadcast_to(b_proj, (P, C)).astype(np.float32)
    )
    ones = np.ones((P, H), dtype=bf16)
    in_maps = []
    for b in range(B):
        in_maps.append(
            {
                "xT": np.ascontiguousarray(x[b].T.astype(bf16)),
                "wqkT": wqkT,
                "wvT": wvT,
                "wpT": wpT,
                "bias_bc": bias_bc,
                "ones_col": ones,
            }
        )
    return in_maps


_CACHED_NC = None


def kernel(x, w_qkv, w_proj, b_proj):
    global _CACHED_NC
    x = np.asarray(x, dtype=np.float32)
    w_qkv = np.asarray(w_qkv, dtype=np.float32)
    w_proj = np.asarray(w_proj, dtype=np.float32)
    b_proj = np.asarray(b_proj, dtype=np.float32)
    if _CACHED_NC is None:
        _CACHED_NC = build_module()
    nc = _CACHED_NC
    in_maps = make_in_maps(x, w_qkv, w_proj, b_proj)
    res = bass_utils.run_bass_kernel_spmd(nc, in_maps, core_ids=list(range(B)))
    return np.stack([res.results[b]["out"] for b in range(B)], axis=0)


if __name__ == "__main__":
    nc = build_module()
    ninst = sum(len(b.instructions) for b in nc.m.functions[0].blocks)
    print("module built ok;", ninst, "instructions")
